# revision 1
# baseline (speedup 1.0000x reference)
"""Causal self-attention (B=2, T=2048, C=1024, H=16) on 8 trn2 NeuronCores.

Sharding: tensor-parallel over heads. Each core owns 2 heads (128 features):
  - qkv projection of the full sequence onto its 384 w_attn columns
  - causal attention for its 2 heads (both batches)
  - partial c_proj: y_local [4096,128] @ w_proj[rows] -> [4096,1024] partial
The 8 partial outputs are summed on the host (the "all-reduce after c_proj"),
plus b_proj.

Everything on-device runs in fp32r (TensorE fast fp32 mode, ~12 mantissa
bits, fp32 PSUM accumulation). End-to-end absmax-relative error vs the fp32
reference is ~8e-4 (measured via numpy simulation of fp32r rounding).

Layout trick: x is transposed on the host (x^T [1024, 4096]) so the qkv
projection consumes it directly as the moving operand; Q^T/K^T come out in
[feature, token] layout, which feeds the S^T = K^T-stationary attention
matmul with softmax denominators obtained from an extra ones-column in V.
"""

import sys

sys.path.insert(0, "/opt/trn_rl_repo")

import numpy as np

N_CORES = 8
B, T, C = 2, 2048, 1024
H, D = 16, 64
HPC = H // N_CORES            # heads per core
F = HPC * D                   # local feature width = 128
BT = B * T                    # 4096 tokens
TCH = 512                     # token chunk (moving-operand width)
NCH = BT // TCH               # 8 token chunks
KB = 128                      # kv block size
NQC = T // TCH                # 4 query chunks per batch

_COMPILED = {}


def _build():
    import concourse.bass as bass
    import concourse.mybir as mybir
    import concourse.tile as tile
    from concourse import bacc

    f32, f32r = mybir.dt.float32, mybir.dt.float32r
    Exp = mybir.ActivationFunctionType.Exp

    nc = bacc.Bacc("TRN2", target_bir_lowering=False, debug=False,
                   num_devices=N_CORES)

    xt = nc.dram_tensor("xt", [C, BT], f32r, kind="ExternalInput")
    wqkv = nc.dram_tensor("wqkv", [C // 128, 128, 3 * F], f32r,
                          kind="ExternalInput")
    bqkv = nc.dram_tensor("bqkv", [F, 3], f32, kind="ExternalInput")
    wp = nc.dram_tensor("wp", [F, C], f32r, kind="ExternalInput")
    tri = nc.dram_tensor("tri", [KB, KB], f32r, kind="ExternalInput")
    eye = nc.dram_tensor("eye", [128, 128], f32, kind="ExternalInput")
    ones = nc.dram_tensor("ones", [128, 64], f32r, kind="ExternalInput")
    out = nc.dram_tensor("out", [BT, C], f32, kind="ExternalOutput")

    with tile.TileContext(nc) as tc, \
         nc.allow_low_precision(reason="fp32r matmul pipeline, fp32 psum"):
        with tc.tile_pool(name="const", bufs=1) as cpool, \
             tc.tile_pool(name="seq", bufs=1) as seq, \
             tc.tile_pool(name="work", bufs=4) as work, \
             tc.tile_pool(name="psBig", bufs=2, space="PSUM") as psBig, \
             tc.tile_pool(name="psS", bufs=2, space="PSUM") as psS, \
             tc.tile_pool(name="psY", bufs=2, space="PSUM") as psY, \
             tc.tile_pool(name="dstage", bufs=4, space="DRAM") as dpool:

            # ---- resident constants ----
            w_sb = cpool.tile([128, C // 128, 3 * F], f32r)
            nc.sync.dma_start(w_sb[:], wqkv.rearrange("a p f -> p a f"))
            b_sb = cpool.tile([F, 3], f32)
            nc.sync.dma_start(b_sb[:], bqkv[:])
            wp_sb = cpool.tile([F, C], f32r)
            nc.gpsimd.dma_start(wp_sb[:], wp[:])
            tri_sb = cpool.tile([KB, KB], f32r)
            nc.gpsimd.dma_start(tri_sb[:], tri[:])
            eye_sb = cpool.tile([128, 128], f32)
            nc.gpsimd.dma_start(eye_sb[:], eye[:])
            ones_sb = cpool.tile([128, 64], f32r)
            nc.gpsimd.dma_start(ones_sb[:], ones[:])

            # ---- resident sequence tensors (per 512-token chunk tiles) ----
            qt_t = [seq.tile([F, TCH], f32r, tag=f"qt{t}", name=f"qt{t}") for t in range(NCH)]
            kt_t = [seq.tile([F, TCH], f32r, tag=f"kt{t}", name=f"kt{t}") for t in range(NCH)]
            # v65[:, i, 0:65] = [V_headA | 1], v65[:, i, 65:130] = [V_headB | 1]
            v65 = seq.tile([128, BT // KB, 130], f32r)
            nc.vector.tensor_copy(
                v65[:, :, 64::65],
                ones_sb[:, 0:64].rearrange("p (a b) -> p a b", b=2))
            yt_t = [seq.tile([F, TCH], f32r, tag=f"yt{t}", name=f"yt{t}") for t in range(NCH)]

            # ---- pre-zeroed diagonal P tiles (paired heads: [128,1024]) ----
            # halves: h0 cols [0:512), h1 cols [512:1024). For a diagonal
            # block with offset r, cols [0:128r) of each half are always
            # zero; zero them once, never rewrite.
            p_diag = {}
            for r in (1, 2, 3):
                for j in (0, 1):
                    pt = seq.tile([128, 2 * TCH], f32r, name=f"pdiag{r}_{j}")
                    pt3 = pt[:].rearrange("p (a q) -> p a q", a=2)
                    nc.vector.memset(pt3[:, :, 0:128 * r].bitcast(f32), 0.0)
                    p_diag[(r, j)] = pt

            # per-(b,bq) denominator tiles in wide layout [16, 64]:
            # rows 8h+p hold den[q = 64p + c] for head h
            dst_tiles = {}
            den_w = {}
            for b in range(B):
                for bq in range(NQC):
                    den_w[(b, bq)] = seq.tile([16, 64], f32,
                                              name=f"denw{b}{bq}")

            def qkv_chunk_gen(t):
                """qkv projection + V transpose for one 512-token chunk.
                Part-outer: one PSUM accumulator live at a time."""
                xts = [work.tile([128, TCH], f32r, tag="xt", bufs=16,
                                 name=f"xts{t}_{i}") for i in range(8)]
                for cb in range(8):
                    nc.gpsimd.dma_start(
                        xts[cb][:],
                        xt[cb * 128:(cb + 1) * 128, t * TCH:(t + 1) * TCH])
                vt_tmp = None
                for part in range(3):
                    ps = psBig.tile([128, TCH], f32, tag="big",
                                    name=f"pqkv{t}_{part}")
                    for cb in range(8):
                        nc.tensor.matmul(
                            ps[:], w_sb[:, cb, part * F:(part + 1) * F],
                            xts[cb][:], start=(cb == 0), stop=(cb == 7))
                        if cb == 3:
                            yield
                    if part == 0:
                        nc.vector.tensor_scalar_add(qt_t[t][:], ps[:],
                                                    b_sb[:, 0:1])
                    elif part == 1:
                        nc.vector.tensor_scalar_add(kt_t[t][:], ps[:],
                                                    b_sb[:, 1:2])
                    else:
                        vt_tmp = work.tile([128, TCH], f32, tag="vt",
                                           name=f"vt{t}")
                        nc.vector.tensor_scalar_add(vt_tmp[:], ps[:],
                                                    b_sb[:, 2:3])
                    yield
                ptr = psS.tile([128, TCH], f32, tag="s", name=f"ptr{t}")
                for i in range(4):
                    nc.tensor.transpose(ptr[:, i * 128:(i + 1) * 128],
                                        vt_tmp[:, i * 128:(i + 1) * 128],
                                        eye_sb[:])
                    if i == 1:
                        yield
                ptr3 = ptr[:].rearrange("p (a k) -> p a k", k=128)
                t4 = t * 4
                nc.vector.tensor_copy(v65[:, t4:t4 + 4, 0:64],
                                      ptr3[:, :, 0:64])
                nc.vector.tensor_copy(v65[:, t4:t4 + 4, 65:129],
                                      ptr3[:, :, 64:128])
                yield

            def norm_proj_gen(b, bq):
                """Per-chunk softmax normalization + projection."""
                qchunk = b * NQC + bq
                last = (b == B - 1 and bq == NQC - 1)
                if not last:
                    rec_w = work.tile([16, 64], f32r, tag="rec",
                                      name=f"rec{b}{bq}")
                    nc.vector.reciprocal(rec_w[:], den_w[(b, bq)][:])
                for h in range(HPC):
                    hs = h * 64
                    rst = work.tile([1, TCH], f32r, tag="rst",
                                    name=f"rst{b}{bq}{h}")
                    if last:
                        # tail chunk: skip the DRAM-roundtrip wide layout;
                        # a direct 1-lane reciprocal is lower latency here
                        nc.vector.reciprocal(rst[:], dst_tiles[(b, bq, h)][:])
                    else:
                        rr = dpool.tile([1, TCH], f32r, tag="rr",
                                        name=f"rr{b}{bq}{h}")
                        nc.gpsimd.dma_start(
                            rr[:].rearrange("o (p c) -> (o p) c", c=64),
                            rec_w[8 * h:8 * h + 8, :])
                        nc.gpsimd.dma_start(rst[:], rr[:])
                    bcast = work.tile([128, TCH], f32r, tag="bcast",
                                      name=f"bcast{b}{bq}{h}")
                    nc.gpsimd.partition_broadcast(bcast[:], rst[:])
                    nc.vector.tensor_mul(yt_t[qchunk][hs:hs + 64, :],
                                         yt_t[qchunk][hs:hs + 64, :],
                                         bcast[hs:hs + 64, :])
                yield
                for ic in range(4):
                    tt = qchunk * 4 + ic
                    for cc in range(2):
                        pj = psBig.tile([128, TCH], f32, tag="big",
                                        name=f"pj{tt}_{cc}")
                        nc.tensor.matmul(
                            pj[:],
                            yt_t[qchunk][:, ic * 128:(ic + 1) * 128],
                            wp_sb[:, cc * TCH:(cc + 1) * TCH],
                            start=True, stop=True)
                        ost = work.tile([128, TCH], f32, tag="ost",
                                        name=f"ost{tt}_{cc}")
                        if (ic + cc) % 2 == 0:
                            nc.scalar.copy(ost[:], pj[:])
                        else:
                            nc.vector.tensor_copy(ost[:], pj[:])
                        nc.sync.dma_start(
                            out[tt * 128:(tt + 1) * 128,
                                cc * TCH:(cc + 1) * TCH], ost[:])
                        yield

            class Filler:
                def __init__(self):
                    self.gens = []

                def add(self, g):
                    self.gens.append(g)

                def step(self):
                    while self.gens:
                        try:
                            next(self.gens[0])
                            return
                        except StopIteration:
                            self.gens.pop(0)

                def drain(self):
                    while self.gens:
                        for _ in self.gens.pop(0):
                            pass

            def attn_pair(b, bq, bk, use_idx):
                """S for both heads into one [128,1024] psum tile + one exp.
                Returns the P tile (halves = heads)."""
                qchunk = b * NQC + bq
                kchunk = b * NQC + bk // 4
                kcol = (bk % 4) * 128
                s_ps = psS.tile([128, 2 * TCH], f32, tag="s",
                                name=f"s{b}{bq}{bk}")
                r = bk - 4 * bq
                # masked q-columns [0:128r) can be skipped entirely when the
                # remaining width stays >= 256 (fp32r full-rate threshold)
                trim = 128 * r if r in (1, 2) else 0
                for h in range(HPC):
                    hs = h * 64
                    nc.tensor.matmul(
                        s_ps[:, h * TCH + trim:(h + 1) * TCH],
                        kt_t[kchunk][hs:hs + 64, kcol:kcol + 128],
                        qt_t[qchunk][hs:hs + 64, trim:],
                        start=True, stop=True)
                if r < 0:
                    p_t = work.tile([128, 2 * TCH], f32r, tag="p", bufs=4,
                                    name=f"p{b}{bq}{bk}")
                    nc.scalar.activation(p_t[:], s_ps[:], Exp)
                    return p_t
                if r == 0:
                    p_t = work.tile([128, 2 * TCH], f32r, tag="p", bufs=4,
                                    name=f"p{b}{bq}{bk}")
                    nc.scalar.activation(p_t[:], s_ps[:], Exp)
                else:
                    p_t = p_diag[(r, use_idx % 2)]
                    s3 = s_ps[:].rearrange("p (a q) -> p a q", a=2)
                    p3 = p_t[:].rearrange("p (a q) -> p a q", a=2)
                    nc.scalar.activation(p3[:, :, 128 * r:],
                                         s3[:, :, 128 * r:], Exp)
                for h in range(HPC):
                    c0 = h * TCH + 128 * r
                    nc.vector.tensor_mul(p_t[:, c0:c0 + 128],
                                         p_t[:, c0:c0 + 128], tri_sb[:])
                return p_t

            def attention_chunk(b, bq, fl):
                qchunk = b * NQC + bq
                nblk = 4 * bq + 4
                yt_ps = [psY.tile([65, TCH], f32, tag="yt",
                                  name=f"ytps{b}{bq}{h}")
                         for h in range(HPC)]
                pend = None

                def emit_pv(bk, p_t, stop):
                    vti = b * (T // KB) + bk
                    r = bk - 4 * bq
                    trim = 128 * r if r in (1, 2) else 0
                    for h in range(HPC):
                        nc.tensor.matmul(
                            yt_ps[h][:, trim:], v65[:, vti, 65 * h:65 * h + 65],
                            p_t[:, h * TCH + trim:(h + 1) * TCH],
                            start=(bk == 0), stop=stop)

                for bk in range(nblk):
                    p_t = attn_pair(b, bq, bk, bq)
                    if pend is not None:
                        emit_pv(pend[0], pend[1], stop=False)
                    pend = (bk, p_t)
                    fl.step()
                emit_pv(pend[0], pend[1], stop=True)
                for h in range(HPC):
                    hs = h * 64
                    nc.vector.tensor_copy(yt_t[qchunk][hs:hs + 64, :],
                                          yt_ps[h][0:64, :])
                    dst = work.tile([1, TCH], f32, tag="dst",
                                    name=f"dst{b}{bq}{h}")
                    nc.vector.tensor_copy(dst[:], yt_ps[h][64:65, :])
                    dst_tiles[(b, bq, h)] = dst
                    dd = dpool.tile([1, TCH], f32, tag="dd",
                                    name=f"dd{b}{bq}{h}")
                    nc.gpsimd.dma_start(dd[:], dst[:])
                    nc.gpsimd.dma_start(
                        den_w[(b, bq)][8 * h:8 * h + 8, :],
                        dd[:].rearrange("o (p c) -> (o p) c", c=64))
                fl.step()

            def chain(*gens):
                for g in gens:
                    yield from g

            # ---- schedule ----
            for t in range(NQC):
                for _ in qkv_chunk_gen(t):
                    pass
            fl = Filler()
            fl.add(chain(*[qkv_chunk_gen(t) for t in range(NQC, NCH)]))
            pending_np = []
            for b in range(B):
                for bq in range(NQC):
                    if b == B - 1 and bq == NQC - 1:
                        # last chunk: make all pending norm+proj work
                        # available as filler so only the final chunk's
                        # chain remains at the tail
                        while pending_np:
                            fl.add(pending_np.pop(0))
                    attention_chunk(b, bq, fl)
                    if pending_np:
                        fl.add(pending_np.pop(0))
                    pending_np.append(norm_proj_gen(b, bq))
            for g in pending_np:
                fl.add(g)
            fl.drain()
    nc.compile()
    return nc


def _get_nc():
    if "nc" not in _COMPILED:
        _COMPILED["nc"] = _build()
    return _COMPILED["nc"]


def _prep_in_maps(x, w_attn, b_attn, w_proj):
    x = np.asarray(x, np.float32)
    w_attn = np.asarray(w_attn, np.float32)
    b_attn = np.asarray(b_attn, np.float32)
    w_proj = np.asarray(w_proj, np.float32)

    scale = np.float32(1.0 / np.sqrt(D))
    xt = np.ascontiguousarray(x.reshape(BT, C).T)          # [C, BT]
    # tri[kv, j] = 1 when j >= kv (upper triangular incl diagonal)
    tri = np.ascontiguousarray(np.triu(np.ones((KB, KB), np.float32)))
    eye = np.eye(128, dtype=np.float32)
    ones = np.ones((128, 64), np.float32)

    in_maps = []
    for c in range(N_CORES):
        cols = slice(c * F, (c + 1) * F)
        wq = w_attn[:, cols] * scale
        wk = w_attn[:, C + c * F:C + (c + 1) * F]
        wv = w_attn[:, 2 * C + c * F:2 * C + (c + 1) * F]
        wqkv = np.ascontiguousarray(
            np.concatenate([wq, wk, wv], axis=1).reshape(C // 128, 128, 3 * F))
        bq = b_attn[c * F:(c + 1) * F] * scale
        bk = b_attn[C + c * F:C + (c + 1) * F]
        bv = b_attn[2 * C + c * F:2 * C + (c + 1) * F]
        bqkv = np.ascontiguousarray(np.stack([bq, bk, bv], axis=1))
        wp = np.ascontiguousarray(w_proj[c * F:(c + 1) * F, :])
        in_maps.append({
            "xt": xt, "wqkv": wqkv, "bqkv": bqkv, "wp": wp,
            "tri": tri, "eye": eye, "ones": ones,
        })
    return in_maps


def _run(inputs, trace=False):
    from concourse.bass_utils import run_bass_kernel_spmd

    nc = _get_nc()
    in_maps = _prep_in_maps(inputs["x"], inputs["w_attn"], inputs["b_attn"],
                            inputs["w_proj"])
    res = run_bass_kernel_spmd(nc, in_maps, list(range(N_CORES)), trace=trace)
    b_proj = np.asarray(inputs["b_proj"], np.float32)
    acc = np.zeros((BT, C), np.float64)
    for c in range(N_CORES):
        acc += res.results[c]["out"]
    y = (acc + b_proj).astype(np.float32).reshape(B, T, C)
    return y, res


def kernel(**inputs):
    y, _ = _run(inputs, trace=False)
    return y



# revision 3
# speedup vs baseline: 1.0782x; 1.0782x over previous
"""Causal self-attention (B=2, T=2048, C=1024, H=16) on 8 trn2 NeuronCores.

Sharding: (batch, head-group). Core c owns batch c//4 and heads
[4*(c%4) .. 4*(c%4)+3] (4 heads = 256 features), so each core:
  - loads x for its batch only, transposed + bf16 (x^T [1024, 2048])
  - qkv projection of its 2048 tokens onto its 768 w_attn columns
  - causal attention for its 4 heads (2 head-pairs)
  - partial c_proj: y_local^T [256, 2048] @ w_proj[rows] -> [2048, 1024]
The 4 partial outputs per batch are summed on the host ("all-reduce after
c_proj"), plus b_proj.

Everything on the matmul path is bf16 (1 PE cycle/column, same stream rate
as fp32r but half the LDWEIGHTS cost, so S head-pairs pipeline at full
rate) with fp32 PSUM accumulation. Softmax denominators ride along as a
65th ones-column in V; reciprocals in fp32->bf16.

Layout: x^T [1024, 2048] feeds qkv as moving operand; Q^T/K^T come out
[feature, token]; S^T = K^T-stationary attention matmul with the two heads
of a pair running concurrently on disjoint PE row groups.
"""

import sys

sys.path.insert(0, "/opt/trn_rl_repo")

import numpy as np
import ml_dtypes

BF16 = ml_dtypes.bfloat16

N_CORES = 8
B, T, C = 2, 2048, 1024
H, D = 16, 64
HPC = 4                       # heads per core
NPAIR = 2                     # head pairs per core
F = HPC * D                   # local feature width = 256
TT = T                        # tokens per core = 2048
TCH = 512                     # token chunk (moving-operand width)
NCH = TT // TCH               # 4 token chunks
KB = 128                      # kv block size

_COMPILED = {}


def _build():
    import concourse.bass as bass
    import concourse.mybir as mybir
    import concourse.tile as tile
    from concourse import bacc

    f32, bf16 = mybir.dt.float32, mybir.dt.bfloat16
    Exp = mybir.ActivationFunctionType.Exp

    nc = bacc.Bacc("TRN2", target_bir_lowering=False, debug=False,
                   num_devices=N_CORES)

    xt = nc.dram_tensor("xt", [C, TT], bf16, kind="ExternalInput")
    wqkv = nc.dram_tensor("wqkv", [C // 128, 128, 6 * 128], bf16,
                          kind="ExternalInput")
    bqkv = nc.dram_tensor("bqkv", [128, 6], f32, kind="ExternalInput")
    wp = nc.dram_tensor("wp", [2, 128, C], bf16, kind="ExternalInput")
    tri = nc.dram_tensor("tri", [KB, KB], bf16, kind="ExternalInput")
    eye = nc.dram_tensor("eye", [128, 128], f32, kind="ExternalInput")
    ones = nc.dram_tensor("ones", [128, 32], bf16, kind="ExternalInput")
    out = nc.dram_tensor("out", [TT, C], bf16, kind="ExternalOutput")

    with tile.TileContext(nc) as tc, \
         nc.allow_low_precision(reason="bf16 matmul pipeline, fp32 psum"):
        with tc.tile_pool(name="const", bufs=1) as cpool, \
             tc.tile_pool(name="seq", bufs=1) as seq, \
             tc.tile_pool(name="work", bufs=4) as work, \
             tc.tile_pool(name="psBig", bufs=2, space="PSUM") as psBig, \
             tc.tile_pool(name="psS", bufs=2, space="PSUM") as psS, \
             tc.tile_pool(name="psY", bufs=2, space="PSUM") as psY:

            # ---- resident constants ----
            w_sb = cpool.tile([128, C // 128, 6 * 128], bf16)
            nc.sync.dma_start(w_sb[:], wqkv.rearrange("a p f -> p a f"))
            b_sb = cpool.tile([128, 6], f32)
            nc.sync.dma_start(b_sb[:], bqkv[:])
            wp_sb = cpool.tile([128, 2, C], bf16)
            nc.gpsimd.dma_start(wp_sb[:], wp.rearrange("a p f -> p a f"))
            tri_sb = cpool.tile([KB, KB], bf16)
            nc.gpsimd.dma_start(tri_sb[:], tri[:])
            eye_sb = cpool.tile([128, 128], f32)
            nc.gpsimd.dma_start(eye_sb[:], eye[:])
            ones_sb = cpool.tile([128, 32], bf16)
            nc.gpsimd.dma_start(ones_sb[:], ones[:])

            # ---- resident sequence tensors, per (pair, 512-token chunk) ----
            qt_t = [[seq.tile([128, TCH], bf16, tag=f"qt{p}{t}",
                              name=f"qt{p}{t}") for t in range(NCH)]
                    for p in range(NPAIR)]
            kt_t = [[seq.tile([128, TCH], bf16, tag=f"kt{p}{t}",
                              name=f"kt{p}{t}") for t in range(NCH)]
                    for p in range(NPAIR)]
            yt_t = [[seq.tile([128, TCH], bf16, tag=f"yt{p}{t}",
                              name=f"yt{p}{t}") for t in range(NCH)]
                    for p in range(NPAIR)]
            # v65[p][:, i, 0:65] = [V_head0 | 1], [:, i, 65:130] = [V_head1 | 1]
            v65 = []
            for p in range(NPAIR):
                v = seq.tile([128, TT // KB, 130], bf16, name=f"v65{p}")
                nc.vector.tensor_copy(
                    v[:, :, 64::65],
                    ones_sb[:, :].rearrange("p (a b) -> p a b", b=2))
                v65.append(v)

            # ---- pre-zeroed diagonal P tiles ([128, 1024], halves=heads) ----
            # for diagonal block offset r, cols [0:128r) of each half are
            # always zero; zero them once, never rewrite.
            p_diag = {}
            for r in (1, 2, 3):
                for j in (0, 1):
                    pt = seq.tile([128, 2 * TCH], bf16, name=f"pdiag{r}_{j}")
                    pt3 = pt[:].rearrange("p (a q) -> p a q", a=2)
                    nc.vector.memset(pt3[:, :, 0:128 * r].bitcast(f32), 0.0)
                    p_diag[(r, j)] = pt

            # per-(bq, h) denominator tiles [1, TCH] fp32 (copied from psum)
            dst_tiles = {}

            def qkv_chunk_gen(t):
                """qkv projection + V transpose for one 512-token chunk."""
                xts = [work.tile([128, TCH], bf16, tag="xt", bufs=16,
                                 name=f"xts{t}_{i}") for i in range(8)]
                for cb in range(8):
                    nc.gpsimd.dma_start(
                        xts[cb][:],
                        xt[cb * 128:(cb + 1) * 128, t * TCH:(t + 1) * TCH])
                vt_tmp = [None, None]
                # slice s covers part s//2 (q,k,v), pair s%2
                for s in range(6):
                    part, pair = s // 2, s % 2
                    ps = psBig.tile([128, TCH], f32, tag="big",
                                    name=f"pqkv{t}_{s}")
                    for cb in range(8):
                        nc.tensor.matmul(
                            ps[:], w_sb[:, cb, s * 128:(s + 1) * 128],
                            xts[cb][:], start=(cb == 0), stop=(cb == 7))
                        if cb == 3:
                            yield
                    if part == 0:
                        nc.vector.tensor_scalar_add(qt_t[pair][t][:], ps[:],
                                                    b_sb[:, s:s + 1])
                    elif part == 1:
                        nc.vector.tensor_scalar_add(kt_t[pair][t][:], ps[:],
                                                    b_sb[:, s:s + 1])
                    else:
                        vt_tmp[pair] = work.tile([128, TCH], f32, tag="vt",
                                                 bufs=2, name=f"vt{t}_{pair}")
                        nc.vector.tensor_scalar_add(vt_tmp[pair][:], ps[:],
                                                    b_sb[:, s:s + 1])
                    yield
                for pair in range(NPAIR):
                    ptr = psS.tile([128, TCH], f32, tag="s",
                                   name=f"ptr{t}_{pair}")
                    for i in range(4):
                        nc.tensor.transpose(ptr[:, i * 128:(i + 1) * 128],
                                            vt_tmp[pair][:, i * 128:(i + 1) * 128],
                                            eye_sb[:])
                        if i == 1:
                            yield
                    ptr3 = ptr[:].rearrange("p (a k) -> p a k", k=128)
                    t4 = t * 4
                    nc.vector.tensor_copy(v65[pair][:, t4:t4 + 4, 0:64],
                                          ptr3[:, :, 0:64])
                    nc.vector.tensor_copy(v65[pair][:, t4:t4 + 4, 65:129],
                                          ptr3[:, :, 64:128])
                    yield

            def norm_proj_gen(bq):
                """Softmax normalization + c_proj for one 512-token chunk."""
                for h in range(HPC):
                    pair, hh = h // 2, h % 2
                    rec = work.tile([1, TCH], bf16, tag="rec",
                                    name=f"rec{bq}{h}")
                    nc.vector.reciprocal(rec[:], dst_tiles[(bq, h)][:])
                    bcast = work.tile([128, TCH], bf16, tag="bcast", bufs=2,
                                      name=f"bcast{bq}{h}")
                    nc.gpsimd.partition_broadcast(bcast[:], rec[:])
                    hs = hh * 64
                    nc.vector.tensor_mul(yt_t[pair][bq][hs:hs + 64, :],
                                         yt_t[pair][bq][hs:hs + 64, :],
                                         bcast[hs:hs + 64, :])
                    if h % 2 == 1:
                        yield
                for ic in range(4):
                    tt_i = bq * 4 + ic
                    for cc in range(2):
                        pj = psBig.tile([128, TCH], f32, tag="big",
                                        name=f"pj{tt_i}_{cc}")
                        for pair in range(NPAIR):
                            nc.tensor.matmul(
                                pj[:],
                                yt_t[pair][bq][:, ic * 128:(ic + 1) * 128],
                                wp_sb[:, pair, cc * TCH:(cc + 1) * TCH],
                                start=(pair == 0), stop=(pair == 1))
                        ost = work.tile([128, TCH], bf16, tag="ost",
                                        name=f"ost{tt_i}_{cc}")
                        if (ic + cc) % 2 == 0:
                            nc.scalar.copy(ost[:], pj[:])
                        else:
                            nc.vector.tensor_copy(ost[:], pj[:])
                        nc.sync.dma_start(
                            out[tt_i * 128:(tt_i + 1) * 128,
                                cc * TCH:(cc + 1) * TCH], ost[:])
                        yield

            class Filler:
                def __init__(self):
                    self.gens = []

                def add(self, g):
                    self.gens.append(g)

                def step(self):
                    while self.gens:
                        try:
                            next(self.gens[0])
                            return
                        except StopIteration:
                            self.gens.pop(0)

                def drain(self):
                    while self.gens:
                        for _ in self.gens.pop(0):
                            pass

            def attn_pair(pair, bq, bk, use_idx):
                """S for a head pair into one [128,1024] psum tile + one exp.
                The two heads' S matmuls run concurrently on PE row groups
                0:64 / 64:128. Returns the P tile (halves = heads)."""
                kchunk = bk // 4
                kcol = (bk % 4) * 128
                s_ps = psS.tile([128, 2 * TCH], f32, tag="s",
                                name=f"s{pair}{bq}{bk}")
                r = bk - 4 * bq
                trim = 128 * r if r > 0 else 0
                for hh in range(2):
                    hs = hh * 64
                    nc.tensor.matmul(
                        s_ps[:, hh * TCH + trim:(hh + 1) * TCH],
                        kt_t[pair][kchunk][hs:hs + 64, kcol:kcol + 128],
                        qt_t[pair][bq][hs:hs + 64, trim:],
                        start=True, stop=True)
                if r < 0:
                    p_t = work.tile([128, 2 * TCH], bf16, tag="p", bufs=4,
                                    name=f"p{pair}{bq}{bk}")
                    nc.scalar.activation(p_t[:], s_ps[:], Exp)
                    return p_t
                if r == 0:
                    p_t = work.tile([128, 2 * TCH], bf16, tag="p", bufs=4,
                                    name=f"p{pair}{bq}{bk}")
                    nc.scalar.activation(p_t[:], s_ps[:], Exp)
                else:
                    p_t = p_diag[(r, use_idx % 2)]
                    s3 = s_ps[:].rearrange("p (a q) -> p a q", a=2)
                    p3 = p_t[:].rearrange("p (a q) -> p a q", a=2)
                    nc.scalar.activation(p3[:, :, 128 * r:],
                                         s3[:, :, 128 * r:], Exp)
                for hh in range(2):
                    c0 = hh * TCH + 128 * r
                    nc.vector.tensor_mul(p_t[:, c0:c0 + 128],
                                         p_t[:, c0:c0 + 128], tri_sb[:])
                return p_t

            def attention_pair_chunk(pair, bq, fl):
                nblk = 4 * bq + 4
                yt_ps = [psY.tile([65, TCH], f32, tag="yt",
                                  name=f"ytps{pair}{bq}{hh}")
                         for hh in range(2)]
                pend = None

                def emit_pv(bk, p_t, stop):
                    r = bk - 4 * bq
                    trim = 128 * r if r > 0 else 0
                    for hh in range(2):
                        nc.tensor.matmul(
                            yt_ps[hh][:, trim:],
                            v65[pair][:, bk, 65 * hh:65 * hh + 65],
                            p_t[:, hh * TCH + trim:(hh + 1) * TCH],
                            start=(bk == 0), stop=stop)

                for bk in range(nblk):
                    p_t = attn_pair(pair, bq, bk, bq + pair)
                    if pend is not None:
                        emit_pv(pend[0], pend[1], stop=False)
                    pend = (bk, p_t)
                    fl.step()
                emit_pv(pend[0], pend[1], stop=True)
                for hh in range(2):
                    h = pair * 2 + hh
                    hs = hh * 64
                    nc.vector.tensor_copy(yt_t[pair][bq][hs:hs + 64, :],
                                          yt_ps[hh][0:64, :])
                    dst = work.tile([1, TCH], f32, tag="dst",
                                    name=f"dst{bq}{h}")
                    nc.vector.tensor_copy(dst[:], yt_ps[hh][64:65, :])
                    dst_tiles[(bq, h)] = dst
                fl.step()

            def chain(*gens):
                for g in gens:
                    yield from g

            # ---- schedule ----
            for _ in qkv_chunk_gen(0):
                pass
            fl = Filler()
            fl.add(chain(*[qkv_chunk_gen(t) for t in range(1, NCH)]))
            pending_np = []
            for bq in range(NCH):
                if bq == NCH - 1:
                    # last chunk: make all pending norm+proj work available
                    # as filler so only the final chunk's chain remains
                    while pending_np:
                        fl.add(pending_np.pop(0))
                for pair in range(NPAIR):
                    attention_pair_chunk(pair, bq, fl)
                if pending_np:
                    fl.add(pending_np.pop(0))
                pending_np.append(norm_proj_gen(bq))
            for g in pending_np:
                fl.add(g)
            fl.drain()
    nc.compile()
    return nc


def _get_nc():
    if "nc" not in _COMPILED:
        _COMPILED["nc"] = _build()
    return _COMPILED["nc"]


def _prep_in_maps(x, w_attn, b_attn, w_proj):
    x = np.asarray(x, np.float32)
    w_attn = np.asarray(w_attn, np.float32)
    b_attn = np.asarray(b_attn, np.float32)
    w_proj = np.asarray(w_proj, np.float32)

    scale = np.float32(1.0 / np.sqrt(D))
    # tri[kv, j] = 1 when j >= kv (upper triangular incl diagonal)
    tri = np.ascontiguousarray(np.triu(np.ones((KB, KB), np.float32))
                               ).astype(BF16)
    eye = np.eye(128, dtype=np.float32)
    ones = np.ones((128, 32), np.float32).astype(BF16)

    xts = [np.ascontiguousarray(x[b].T).astype(BF16) for b in range(B)]

    in_maps = []
    for c in range(N_CORES):
        b, hg = c // 4, c % 4
        f0 = hg * F
        slices_w, slices_b = [], []
        for part in range(3):
            for pair in range(NPAIR):
                lo = part * C + f0 + pair * 128
                wsl = w_attn[:, lo:lo + 128]
                bsl = b_attn[lo:lo + 128]
                if part == 0:
                    wsl = wsl * scale
                    bsl = bsl * scale
                slices_w.append(wsl)
                slices_b.append(bsl)
        wqkv = np.ascontiguousarray(
            np.concatenate(slices_w, axis=1).reshape(C // 128, 128, 6 * 128)
        ).astype(BF16)
        bqkv = np.ascontiguousarray(np.stack(slices_b, axis=1))
        wpc = np.ascontiguousarray(
            w_proj[f0:f0 + F, :].reshape(2, 128, C)).astype(BF16)
        in_maps.append({
            "xt": xts[b], "wqkv": wqkv, "bqkv": bqkv, "wp": wpc,
            "tri": tri, "eye": eye, "ones": ones,
        })
    return in_maps


def _run(inputs, trace=False):
    from concourse.bass_utils import run_bass_kernel_spmd

    nc = _get_nc()
    in_maps = _prep_in_maps(inputs["x"], inputs["w_attn"], inputs["b_attn"],
                            inputs["w_proj"])
    res = run_bass_kernel_spmd(nc, in_maps, list(range(N_CORES)), trace=trace)
    b_proj = np.asarray(inputs["b_proj"], np.float32)
    y = np.zeros((B, TT, C), np.float32)
    for b in range(B):
        acc = np.zeros((TT, C), np.float64)
        for hg in range(4):
            acc += np.asarray(res.results[b * 4 + hg]["out"], np.float64)
        y[b] = (acc + b_proj).astype(np.float32)
    return y, res


def kernel(**inputs):
    y, _ = _run(inputs, trace=False)
    return y


# revision 13
# speedup vs baseline: 1.3406x; 1.2434x over previous
"""Causal self-attention (B=2, T=2048, C=1024, H=16) on 8 trn2 NeuronCores.

Sharding: (batch, head-group). Core c owns batch c//4 and heads
[4*(c%4) .. 4*(c%4)+3] (4 heads = 256 features), so each core:
  - loads x for its batch only, transposed + bf16 (x^T [1024, 2048])
  - qkv projection of its 2048 tokens onto its 768 w_attn columns
  - causal attention for its 4 heads (2 head-pairs)
  - partial c_proj: y_local^T [256, 2048] @ w_proj[rows] -> [2048, 1024]
The 4 partial outputs per batch are summed on the host ("all-reduce after
c_proj"), plus the effective bias.

Bias folding (host): softmax is invariant to per-query-column constants,
so the K bias drops entirely; the V bias passes through softmax as a
constant (rows sum to 1) and folds into b_proj as b_v @ w_proj. Only the
Q bias stays in-kernel.

Everything on the matmul path is bf16 (1 PE cycle/column, same stream rate
as fp32r but half the LDWEIGHTS cost, so S head-pairs run concurrently on
disjoint PE row groups at full rate) with fp32 PSUM accumulation. Softmax
denominators ride along as a 65th ones-column in V; reciprocals via the
batched fast-approx DVE op (the exact `reciprocal` is ~8 cyc/elem and was
the previous bottleneck).
"""

import sys

sys.path.insert(0, "/opt/trn_rl_repo")

import numpy as np
import ml_dtypes

BF16 = ml_dtypes.bfloat16

N_CORES = 8
B, T, C = 2, 2048, 1024
H, D = 16, 64
HPC = 4                       # heads per core
NPAIR = 2                     # head pairs per core
F = HPC * D                   # local feature width = 256
TT = T                        # tokens per core = 2048
TCH = 512                     # token chunk (moving-operand width)
NCH = TT // TCH               # 4 token chunks
KB = 128                      # kv block size

_COMPILED = {}


def _build():
    import concourse.bass as bass
    import concourse.mybir as mybir
    import concourse.tile as tile
    from concourse import bacc

    f32, bf16 = mybir.dt.float32, mybir.dt.bfloat16
    Exp = mybir.ActivationFunctionType.Exp

    nc = bacc.Bacc("TRN2", target_bir_lowering=False, debug=False,
                   num_devices=N_CORES)

    xt = nc.dram_tensor("xt", [C, TT], bf16, kind="ExternalInput")
    wqkv = nc.dram_tensor("wqkv", [C // 128, 128, 6 * 128], bf16,
                          kind="ExternalInput")
    bq2 = nc.dram_tensor("bq2", [128, 2], f32, kind="ExternalInput")
    wp = nc.dram_tensor("wp", [2, 128, C], bf16, kind="ExternalInput")
    tri2 = nc.dram_tensor("tri2", [KB, 2, KB], bf16, kind="ExternalInput")
    eye = nc.dram_tensor("eye", [128, 128], bf16, kind="ExternalInput")
    ones = nc.dram_tensor("ones", [128, 32], bf16, kind="ExternalInput")
    out = nc.dram_tensor("out", [TT, C], bf16, kind="ExternalOutput")

    with tile.TileContext(nc) as tc, \
         nc.allow_low_precision(reason="bf16 matmul pipeline, fp32 psum"):
        with tc.tile_pool(name="const", bufs=1) as cpool, \
             tc.tile_pool(name="seq", bufs=1) as seq, \
             tc.tile_pool(name="work", bufs=4) as work, \
             tc.tile_pool(name="psBig", bufs=2, space="PSUM") as psBig, \
             tc.tile_pool(name="psS", bufs=2, space="PSUM") as psS, \
             tc.tile_pool(name="psY", bufs=2, space="PSUM") as psY:

            # ---- first-chunk x DMAs before everything else (prologue) ----
            xts0 = [work.tile([128, TCH], bf16, tag="xt", bufs=16,
                              name=f"xts0_{i}") for i in range(8)]
            qeng = [nc.sync, nc.scalar, nc.gpsimd]
            for cb in range(8):
                qeng[cb % 3].dma_start(xts0[cb][:],
                                       xt[cb * 128:(cb + 1) * 128, 0:TCH])

            # ---- resident constants (w split per contract block so the
            #      first qkv matmuls can start before the full transfer) ----
            w_sb = cpool.tile([128, C // 128, 6 * 128], bf16)
            for cb in range(8):
                qeng[cb % 3].dma_start(w_sb[:, cb, :], wqkv[cb])
            b_sb = cpool.tile([128, 2], f32)
            nc.sync.dma_start(b_sb[:], bq2[:])
            wp_sb = cpool.tile([128, 2, C], bf16)
            nc.gpsimd.dma_start(wp_sb[:], wp.rearrange("a p f -> p a f"))
            tri_sb = cpool.tile([KB, 2, KB], bf16)
            nc.gpsimd.dma_start(tri_sb[:], tri2[:])
            eye_sb = cpool.tile([128, 128], bf16)
            nc.gpsimd.dma_start(eye_sb[:], eye[:])
            ones_sb = cpool.tile([128, 32], bf16)
            nc.gpsimd.dma_start(ones_sb[:], ones[:])

            # ---- resident sequence tensors, per (pair, 512-token chunk) ----
            qt_t = [[seq.tile([128, TCH], bf16, tag=f"qt{p}{t}",
                              name=f"qt{p}{t}") for t in range(NCH)]
                    for p in range(NPAIR)]
            kt_t = [[seq.tile([128, TCH], bf16, tag=f"kt{p}{t}",
                              name=f"kt{p}{t}") for t in range(NCH)]
                    for p in range(NPAIR)]
            yt_t = [[seq.tile([128, TCH], bf16, tag=f"yt{p}{t}",
                              name=f"yt{p}{t}") for t in range(NCH)]
                    for p in range(NPAIR)]
            # v65[p][:, i, 0:65] = [V_head0 | 1], [:, i, 65:130] = [V_head1 | 1]
            v65 = []
            for p in range(NPAIR):
                v = seq.tile([128, TT // KB, 130], bf16, name=f"v65{p}")
                nc.vector.tensor_copy(
                    v[:, :, 64::65],
                    ones_sb[:, :].rearrange("p (a b) -> p a b", b=2))
                v65.append(v)

            # ---- pre-zeroed diagonal P tiles ([128, 1024], halves=heads) ----
            p_diag = {}
            for r in (1, 2, 3):
                for j in (0, 1):
                    pt = seq.tile([128, 2 * TCH], bf16, name=f"pdiag{r}_{j}")
                    pt3 = pt[:].rearrange("p (a q) -> p a q", a=2)
                    nc.vector.memset(pt3[:, :, 0:128 * r].bitcast(f32), 0.0)
                    p_diag[(r, j)] = pt

            # per-bq denominator tiles [4, TCH] fp32 (rows = heads)
            den_tiles = {}

            def qkv_chunk_gen(t, xts=None):
                """qkv projection + V transpose for one 512-token chunk."""
                if xts is None:
                    xts = [work.tile([128, TCH], bf16, tag="xt", bufs=16,
                                     name=f"xts{t}_{i}") for i in range(8)]
                    dq = [nc.sync, nc.gpsimd]
                    for cb in range(8):
                        dq[cb % 2].dma_start(
                            xts[cb][:],
                            xt[cb * 128:(cb + 1) * 128,
                               t * TCH:(t + 1) * TCH])
                vt_tmp = [None, None]
                # slice s covers part s//2 (q,k,v), pair s%2
                for s in range(6):
                    part, pair = s // 2, s % 2
                    ps = psBig.tile([128, TCH], f32, tag="big",
                                    name=f"pqkv{t}_{s}")
                    for cb in range(8):
                        nc.tensor.matmul(
                            ps[:], w_sb[:, cb, s * 128:(s + 1) * 128],
                            xts[cb][:], start=(cb == 0), stop=(cb == 7))
                        if cb == 3:
                            yield
                    if part == 0:
                        # only Q keeps a bias (K's drops under softmax
                        # shift-invariance, V's is folded into b_proj)
                        if t == 0:
                            nc.scalar.add(qt_t[pair][t][:], ps[:],
                                          b_sb[:, s:s + 1])
                        else:
                            nc.vector.tensor_scalar_add(qt_t[pair][t][:],
                                                        ps[:],
                                                        b_sb[:, s:s + 1])
                    elif part == 1:
                        nc.vector.tensor_copy(kt_t[pair][t][:], ps[:])
                    else:
                        vt_tmp[pair] = work.tile([128, TCH], bf16, tag="vt",
                                                 bufs=2, name=f"vt{t}_{pair}")
                        nc.vector.tensor_copy(vt_tmp[pair][:], ps[:])
                    yield
                for pair in range(NPAIR):
                    ptr = psS.tile([128, TCH], bf16, tag="s",
                                   name=f"ptr{t}_{pair}")
                    for i in range(4):
                        nc.tensor.transpose(ptr[:, i * 128:(i + 1) * 128],
                                            vt_tmp[pair][:, i * 128:(i + 1) * 128],
                                            eye_sb[:])
                        if i == 1:
                            yield
                    # single strided copy: [kv, blk, head, d] <- [kv, blk*d]
                    t4 = t * 4
                    nc.vector.tensor_copy(
                        v65[pair][:, t4:t4 + 4, :].rearrange(
                            "p a (h c) -> p a h c", c=65)[:, :, :, 0:64],
                        ptr[:].rearrange("p (a h c) -> p a h c", a=4, h=2))
                    yield

            def norm_proj_gen(bq):
                """Softmax normalization + c_proj for one 512-token chunk."""
                for h in range(HPC):
                    pair, hh = h // 2, h % 2
                    hs = hh * 64
                    rec = work.tile([1, TCH], f32, tag="rec", bufs=4,
                                    name=f"rec{bq}{h}")
                    nc.vector.reciprocal_approx_fast(rec[:],
                                                     den_tiles[(bq, h)][:])
                    bcast = work.tile([128, TCH], f32, tag="bcast", bufs=2,
                                      name=f"bcast{bq}{h}")
                    nc.gpsimd.partition_broadcast(bcast[:], rec[:])
                    nc.vector.tensor_mul(yt_t[pair][bq][hs:hs + 64, :],
                                         yt_t[pair][bq][hs:hs + 64, :],
                                         bcast[hs:hs + 64, :])
                    if hh == 1:
                        yield
                for ic in range(4):
                    tt_i = bq * 4 + ic
                    for cc in range(2):
                        pj = psBig.tile([128, TCH], f32, tag="big",
                                        name=f"pj{tt_i}_{cc}")
                        for pair in range(NPAIR):
                            nc.tensor.matmul(
                                pj[:],
                                yt_t[pair][bq][:, ic * 128:(ic + 1) * 128],
                                wp_sb[:, pair, cc * TCH:(cc + 1) * TCH],
                                start=(pair == 0), stop=(pair == 1))
                        ost = work.tile([128, TCH], bf16, tag="ost",
                                        name=f"ost{tt_i}_{cc}")
                        nc.vector.tensor_copy(ost[:], pj[:])
                        nc.sync.dma_start(
                            out[tt_i * 128:(tt_i + 1) * 128,
                                cc * TCH:(cc + 1) * TCH], ost[:])
                        yield

            class Filler:
                def __init__(self):
                    self.gens = []

                def add(self, g):
                    self.gens.append(g)

                def step(self):
                    while self.gens:
                        try:
                            next(self.gens[0])
                            return
                        except StopIteration:
                            self.gens.pop(0)

                def drain(self):
                    while self.gens:
                        for _ in self.gens.pop(0):
                            pass

            def attn_pair(pair, bq, bk, use_idx):
                """S for a head pair into one [128,1024] psum tile + one exp.
                The two heads' S matmuls run concurrently on PE row groups
                0:64 / 64:128. Returns the P tile (halves = heads)."""
                kchunk = bk // 4
                kcol = (bk % 4) * 128
                s_ps = psS.tile([128, 2 * TCH], f32, tag="s",
                                name=f"s{pair}{bq}{bk}")
                r = bk - 4 * bq
                trim = 128 * r if r > 0 else 0
                for hh in range(2):
                    hs = hh * 64
                    nc.tensor.matmul(
                        s_ps[:, hh * TCH + trim:(hh + 1) * TCH],
                        kt_t[pair][kchunk][hs:hs + 64, kcol:kcol + 128],
                        qt_t[pair][bq][hs:hs + 64, trim:],
                        start=True, stop=True)
                if r < 0:
                    p_t = work.tile([128, 2 * TCH], bf16, tag="p", bufs=4,
                                    name=f"p{pair}{bq}{bk}")
                    nc.scalar.activation(p_t[:], s_ps[:], Exp)
                    return p_t
                if r == 0:
                    p_t = work.tile([128, 2 * TCH], bf16, tag="p", bufs=4,
                                    name=f"p{pair}{bq}{bk}")
                    nc.scalar.activation(p_t[:], s_ps[:], Exp)
                else:
                    p_t = p_diag[(r, use_idx % 2)]
                    s3 = s_ps[:].rearrange("p (a q) -> p a q", a=2)
                    p3 = p_t[:].rearrange("p (a q) -> p a q", a=2)
                    nc.scalar.activation(p3[:, :, 128 * r:],
                                         s3[:, :, 128 * r:], Exp)
                # one strided mask multiply covering both heads
                p3m = p_t[:].rearrange("p (a q) -> p a q", a=2)
                nc.vector.tensor_mul(p3m[:, :, 128 * r:128 * r + 128],
                                     p3m[:, :, 128 * r:128 * r + 128],
                                     tri_sb[:])
                return p_t

            def attention_pair_chunk(pair, bq, fl):
                nblk = 4 * bq + 4
                yt_ps = [psY.tile([65, TCH], f32, tag="yt",
                                  name=f"ytps{pair}{bq}{hh}")
                         for hh in range(2)]
                pend = None

                def emit_pv(bk, p_t, stop):
                    r = bk - 4 * bq
                    trim = 128 * r if r > 0 else 0
                    for hh in range(2):
                        nc.tensor.matmul(
                            yt_ps[hh][:, trim:],
                            v65[pair][:, bk, 65 * hh:65 * hh + 65],
                            p_t[:, hh * TCH + trim:(hh + 1) * TCH],
                            start=(bk == 0), stop=stop)

                for bk in range(nblk):
                    p_t = attn_pair(pair, bq, bk, bq + pair)
                    if pend is not None:
                        emit_pv(pend[0], pend[1], stop=False)
                    pend = (bk, p_t)
                    fl.step()
                emit_pv(pend[0], pend[1], stop=True)
                for hh in range(2):
                    h = pair * 2 + hh
                    hs = hh * 64
                    nc.vector.tensor_copy(yt_t[pair][bq][hs:hs + 64, :],
                                          yt_ps[hh][0:64, :])
                    dst = work.tile([1, TCH], f32, tag="den", bufs=8,
                                    name=f"den{bq}{h}")
                    nc.vector.tensor_copy(dst[:], yt_ps[hh][64:65, :])
                    den_tiles[(bq, h)] = dst
                fl.step()

            def chain(*gens):
                for g in gens:
                    yield from g

            # ---- schedule ----
            for _ in qkv_chunk_gen(0, xts=xts0):
                pass
            fl = Filler()
            fl.add(chain(*[qkv_chunk_gen(t) for t in range(1, NCH)]))
            pending_np = []
            for bq in range(NCH):
                if bq == NCH - 1:
                    while pending_np:
                        fl.add(pending_np.pop(0))
                for pair in range(NPAIR):
                    attention_pair_chunk(pair, bq, fl)
                if pending_np:
                    fl.add(pending_np.pop(0))
                pending_np.append(norm_proj_gen(bq))
            for g in pending_np:
                fl.add(g)
            fl.drain()
    nc.compile()
    return nc


def _get_nc():
    if "nc" not in _COMPILED:
        _COMPILED["nc"] = _build()
    return _COMPILED["nc"]


def _prep_in_maps(x, w_attn, b_attn, w_proj):
    x = np.asarray(x, np.float32)
    w_attn = np.asarray(w_attn, np.float32)
    b_attn = np.asarray(b_attn, np.float32)
    w_proj = np.asarray(w_proj, np.float32)

    scale = np.float32(1.0 / np.sqrt(D))
    tri = np.triu(np.ones((KB, KB), np.float32))
    tri2 = np.ascontiguousarray(np.stack([tri, tri], axis=1)).astype(BF16)
    eye = np.eye(128, dtype=np.float32).astype(BF16)
    ones = np.ones((128, 32), np.float32).astype(BF16)

    xts = [np.ascontiguousarray(x[b].T).astype(BF16) for b in range(B)]

    in_maps = []
    for c in range(N_CORES):
        b, hg = c // 4, c % 4
        f0 = hg * F
        slices_w, slices_bq = [], []
        for part in range(3):
            for pair in range(NPAIR):
                lo = part * C + f0 + pair * 128
                wsl = w_attn[:, lo:lo + 128]
                if part == 0:
                    wsl = wsl * scale
                    slices_bq.append(b_attn[lo:lo + 128] * scale)
                slices_w.append(wsl)
        wqkv = np.ascontiguousarray(
            np.concatenate(slices_w, axis=1).reshape(C // 128, 128, 6 * 128)
        ).astype(BF16)
        bq2 = np.ascontiguousarray(np.stack(slices_bq, axis=1))
        wpc = np.ascontiguousarray(
            w_proj[f0:f0 + F, :].reshape(2, 128, C)).astype(BF16)
        in_maps.append({
            "xt": xts[b], "wqkv": wqkv, "bq2": bq2, "wp": wpc,
            "tri2": tri2, "eye": eye, "ones": ones,
        })
    return in_maps


def _run(inputs, trace=False):
    from concourse.bass_utils import run_bass_kernel_spmd

    nc = _get_nc()
    in_maps = _prep_in_maps(inputs["x"], inputs["w_attn"], inputs["b_attn"],
                            inputs["w_proj"])
    res = run_bass_kernel_spmd(nc, in_maps, list(range(N_CORES)), trace=trace)
    b_attn = np.asarray(inputs["b_attn"], np.float32)
    w_proj = np.asarray(inputs["w_proj"], np.float32)
    # V bias passes through softmax (rows sum to 1) -> fold into b_proj
    b_eff = (np.asarray(inputs["b_proj"], np.float64)
             + b_attn[2 * C:].astype(np.float64) @ w_proj.astype(np.float64))
    y = np.zeros((B, TT, C), np.float32)
    for b in range(B):
        acc = np.zeros((TT, C), np.float64)
        for hg in range(4):
            acc += np.asarray(res.results[b * 4 + hg]["out"], np.float64)
        y[b] = (acc + b_eff).astype(np.float32)
    return y, res


def kernel(**inputs):
    y, _ = _run(inputs, trace=False)
    return y
